# revision 1
# baseline (speedup 1.0000x reference)
"""GQA causal self-attention (B=2, T=2048, C=2048, 16 Q heads / 4 KV heads,
head_dim=128) on 8 TRN2 NeuronCores.

Sharding: core = (batch b, kv-group g) for b in {0,1}, g in {0..3}.
Each core computes its batch's 4 Q heads that share KV head g, plus the
partial out-projection over those heads' rows of W_out. Host sums the 4
partials per batch and adds b_out.

Device layout choices (all feature-major, "T" on the free axis):
  - qT/kT [d=128 part, t free]  -> scores S^T[j,i] = kT_tile.T @ qT_slice
  - softmax over j (= partition axis of S^T): exp on ACT (scale=1/128
    fused), causal mask via gpsimd.affine_select, column-sum via
    ones-vector matmul, broadcast of 1/den back over partitions via a
    rank-1 (K=1) matmul.
  - y^T[d, i] = v_tile.T-free accumulation: lhsT = v[t,d] tiles, rhs = P^T.
  - out[t, e] = y^T as lhsT directly against W_out rows.
All matmul operands bf16 (fp32 PSUM accumulation); everything else fp32.
"""

import sys

if "/opt/trn_rl_repo" not in sys.path:
    sys.path.insert(0, "/opt/trn_rl_repo")

import numpy as np
import ml_dtypes

BF16 = ml_dtypes.bfloat16

B = 2
T = 2048
C = 2048
NH = 16
NKV = 4
D = 128
GQ = NH // NKV  # 4 q heads per kv head
N_CORES = 8
CC = C // 128  # 16 contraction chunks
TS = T // 512  # 4 t-slices
TT = T // 128  # 16 t-tiles
NF = GQ + 2  # feature chunks per core: 4 q heads + k + v

_CACHED = {}


def _build_bass(reps=1):
    import concourse.bass as bass
    import concourse.bacc as bacc
    import concourse.tile as tile
    import concourse.mybir as mybir

    bf = mybir.dt.bfloat16
    f32 = mybir.dt.float32
    Exp = mybir.ActivationFunctionType.Exp

    nc = bacc.Bacc(None, target_bir_lowering=False)

    # DRAM inputs (host pre-laid-out, see kernel())
    xT_d = nc.dram_tensor("xt", [128, CC, T], bf, kind="ExternalInput")
    wqkv_d = nc.dram_tensor("wqkv", [128, CC, NF * 128], bf, kind="ExternalInput")
    bqkv_d = nc.dram_tensor("bqkv", [128, NF], f32, kind="ExternalInput")
    cos_d = nc.dram_tensor("cosT", [128, T], f32, kind="ExternalInput")
    sin_d = nc.dram_tensor("sinT", [128, T], f32, kind="ExternalInput")
    swap_d = nc.dram_tensor("swp", [128, 128], bf, kind="ExternalInput")
    iden_d = nc.dram_tensor("idn", [128, 128], bf, kind="ExternalInput")
    wout_d = nc.dram_tensor("wout", [128, GQ, C], bf, kind="ExternalInput")
    out_d = nc.dram_tensor("out", [T, C], f32, kind="ExternalOutput")

    with tile.TileContext(nc) as tc:
        with (
            tc.tile_pool(name="persist", bufs=1) as pers,
            tc.tile_pool(name="xt", bufs=2) as xtp,
            tc.tile_pool(name="stage", bufs=3) as stg,
            tc.tile_pool(name="ptile", bufs=6) as ptp,
            tc.tile_pool(name="small", bufs=4) as smp,
            tc.tile_pool(name="osb", bufs=3) as osp,
            tc.tile_pool(name="ps_qkv", bufs=2, space="PSUM") as ppq,
            tc.tile_pool(name="ps_sc", bufs=2, space="PSUM") as pps,
            tc.tile_pool(name="ps_y", bufs=2, space="PSUM") as ppy,
            tc.tile_pool(name="ps_d", bufs=2, space="PSUM") as ppd,
        ):
            import contextlib
            loop_cm = tc.For_i(0, reps, 1) if reps > 1 else contextlib.nullcontext()
            with loop_cm:
                _body(nc, tc, mybir, bf, f32, Exp,
                      pers, xtp, stg, ptp, smp, osp, ppq, pps, ppy, ppd,
                      xT_d, wqkv_d, bqkv_d, cos_d, sin_d, swap_d, iden_d, wout_d, out_d)
    nc.compile()
    return nc


def _body(nc, tc, mybir, bf, f32, Exp,
          pers, xtp, stg, ptp, smp, osp, ppq, pps, ppy, ppd,
          xT_d, wqkv_d, bqkv_d, cos_d, sin_d, swap_d, iden_d, wout_d, out_d):
            # ---- persistent loads ----
            wq_sb = pers.tile([128, CC, NF * 128], bf)
            nc.sync.dma_start(wq_sb[:, 0:1, :], wqkv_d[:, 0:1, :])
            xt0 = xtp.tile([128, CC, 512], bf, tag="xt")
            nc.sync.dma_start(xt0[:, 0:1, :], xT_d[:, 0:1, 0:512])
            nc.sync.dma_start(wq_sb[:, 1:4, :], wqkv_d[:, 1:4, :])
            for xc in range(1, 4):
                nc.sync.dma_start(xt0[:, xc * 4 - 3 : xc * 4 + 1, :],
                                  xT_d[:, xc * 4 - 3 : xc * 4 + 1, 0:512])
            nc.sync.dma_start(xt0[:, 13:16, :], xT_d[:, 13:16, 0:512])
            bq_sb = pers.tile([128, NF], f32)
            nc.sync.dma_start(bq_sb[:], bqkv_d[:])
            swap_sb = pers.tile([128, 128], bf)
            nc.sync.dma_start(swap_sb[:], swap_d[:])
            iden_sb = pers.tile([128, 128], bf)
            nc.sync.dma_start(iden_sb[:], iden_d[:])
            for wc in range(1, 4):
                nc.sync.dma_start(wq_sb[:, wc * 4 : (wc + 1) * 4, :],
                                  wqkv_d[:, wc * 4 : (wc + 1) * 4, :])
            cos_sb = pers.tile([128, T], f32)
            nc.sync.dma_start(cos_sb[:], cos_d[:])
            sin_sb = pers.tile([128, T], f32)
            nc.sync.dma_start(sin_sb[:], sin_d[:])
            wout_sb = pers.tile([128, GQ, C], bf)
            nc.sync.dma_start(wout_sb[:], wout_d[:])
            ones_sb = pers.tile([128, 1], bf)
            nc.vector.memset(ones_sb[:], 1.0)

            # persistent activations
            qk_sb = pers.tile([128, GQ + 1, T], bf)  # rotated q0..q3, k
            v_sb = pers.tile([128, TT, 128], bf)  # v in [t-part, d] tiles
            y_sb = pers.tile([128, GQ, T], bf)  # y^T per head

            mul = mybir.AluOpType.mult
            add = mybir.AluOpType.add

            # ---- phase 1: QKV + RoPE + v transpose ----
            for ts in range(TS):
                tsl = slice(ts * 512, (ts + 1) * 512)
                if ts == 0:
                    xt = xt0
                else:
                    xt = xtp.tile([128, CC, 512], bf, tag="xt")
                    nc.sync.dma_start(xt[:], xT_d[:, :, tsl])
                for f in range(NF):
                    ps = ppq.tile([128, 512], f32, tag="qkvps")
                    for cc in range(CC):
                        nc.tensor.matmul(
                            ps[:],
                            wq_sb[:, cc, f * 128 : (f + 1) * 128],
                            xt[:, cc, :],
                            start=(cc == 0),
                            stop=(cc == CC - 1),
                        )
                    # bias add (also PSUM->SBUF move), bf16 out
                    raw = stg.tile([128, 512], bf, tag="raw")
                    nc.vector.tensor_tensor(
                        raw[:], ps[:], bq_sb[:, f : f + 1].to_broadcast((128, 512)), add
                    )
                    if f < NF - 1:
                        # rope: rot = raw*cos + swap(raw)*sinsign
                        psw = pps.tile([128, 512], f32, tag="sps")
                        nc.tensor.matmul(psw[:], swap_sb[:], raw[:], start=True, stop=True)
                        tmp = stg.tile([128, 512], bf, tag="ropetmp")
                        nc.vector.tensor_tensor(tmp[:], psw[:], sin_sb[:, tsl], mul)
                        nc.vector.tensor_tensor(
                            qk_sb[:, f, tsl], raw[:], cos_sb[:, tsl], mul
                        )
                        nc.vector.tensor_tensor(
                            qk_sb[:, f, tsl], qk_sb[:, f, tsl], tmp[:], add
                        )
                    else:
                        # v: transpose [d, t] -> [t, d] via PE
                        for k in range(4):
                            pst = pps.tile([128, 128], bf, tag="sps")
                            nc.tensor.transpose(
                                pst[:], raw[:, k * 128 : (k + 1) * 128], iden_sb[:]
                            )
                            nc.any.tensor_copy(v_sb[:, ts * 4 + k, :], pst[:])

            # ---- phase 2+3: per i-slice: attention (4 heads) then out-proj ----
            for s in range(TS):
                isl = slice(s * 512, (s + 1) * 512)
                njt = 4 * (s + 1)
                for h in range(GQ):
                    psy = ppy.tile([128, 512], f32, tag="yps")
                    psd = ppd.tile([1, 512], f32, tag="dps")
                    for jt in range(njt):
                        # columns i < 128*jt are fully masked: skip them
                        off = max(0, 128 * jt - 512 * s)
                        pss = pps.tile([128, 512], f32, tag="sps")
                        nc.tensor.matmul(
                            pss[:, off:512],
                            qk_sb[:, GQ, jt * 128 : (jt + 1) * 128],
                            qk_sb[:, h, s * 512 + off : (s + 1) * 512],
                            start=True,
                            stop=True,
                        )
                        P = ptp.tile([128, 512], bf, tag="P")
                        nc.scalar.activation(
                            P[:, off:512], pss[:, off:512], Exp, scale=1.0 / 128.0
                        )
                        if jt >= 4 * s:
                            # triangular block: keep where p <= y (y rel. to off)
                            nc.gpsimd.affine_select(
                                out=P[:, off : off + 128],
                                in_=P[:, off : off + 128],
                                pattern=[[1, 128]],
                                compare_op=mybir.AluOpType.is_ge,
                                fill=0.0,
                                base=0,
                                channel_multiplier=-1,
                            )
                        nc.tensor.matmul(
                            psy[:, off:512],
                            v_sb[:, jt, :],
                            P[:, off:512],
                            start=(jt == 0),
                            stop=(jt == njt - 1),
                        )
                        nc.tensor.matmul(
                            psd[:, off:512],
                            ones_sb[:],
                            P[:, off:512],
                            start=(jt == 0),
                            stop=(jt == njt - 1),
                        )
                    rden = smp.tile([1, 512], f32, tag="rden")
                    nc.vector.reciprocal(rden[:], psd[:])
                    rdb = smp.tile([128, 512], f32, tag="rdb")
                    nc.gpsimd.partition_broadcast(rdb[:], rden[:])
                    nc.vector.tensor_tensor(y_sb[:, h, isl], psy[:], rdb[:], mul)

                for tt in range(4 * s, 4 * s + 4):
                    o_sb = osp.tile([128, C], f32, tag="osb")
                    for es in range(4):
                        pso = ppy.tile([128, 512], f32, tag="yps")
                        for h in range(GQ):
                            nc.tensor.matmul(
                                pso[:],
                                y_sb[:, h, tt * 128 : (tt + 1) * 128],
                                wout_sb[:, h, es * 512 : (es + 1) * 512],
                                start=(h == 0),
                                stop=(h == GQ - 1),
                            )
                        if es % 2 == 0:
                            nc.vector.tensor_copy(
                                o_sb[:, es * 512 : (es + 1) * 512], pso[:]
                            )
                        else:
                            nc.scalar.copy(
                                o_sb[:, es * 512 : (es + 1) * 512], pso[:]
                            )
                    nc.sync.dma_start(out_d[tt * 128 : (tt + 1) * 128, :], o_sb[:])


def _host_prep(x, rope_cache, W_qkv, b_qkv, W_out):
    """Build the 8 per-core input dicts."""
    q_dim = NH * D  # 2048
    kv_dim = NKV * D  # 512

    # rope tables in [d, t] layout
    sin = rope_cache[:, 0::2].astype(np.float32)  # [T, 64]
    cos = rope_cache[:, 1::2].astype(np.float32)
    cos2T = np.empty((128, T), np.float32)
    sinsT = np.empty((128, T), np.float32)
    cos2T[0::2] = cos.T
    cos2T[1::2] = cos.T
    sinsT[0::2] = -sin.T
    sinsT[1::2] = sin.T

    swap = np.zeros((128, 128), BF16)
    idx = np.arange(128)
    swap[idx, idx ^ 1] = 1
    iden = np.eye(128, dtype=BF16)

    in_maps = []
    for b in range(B):
        xT = np.ascontiguousarray(x[b].T.astype(BF16))  # [C, T]
        xT = xT.reshape(CC, 128, T).transpose(1, 0, 2)  # [128, CC, T]
        xT = np.ascontiguousarray(xT)
        for g in range(NKV):
            cols = np.concatenate(
                [
                    np.arange(4 * g * D, (4 * g + 4) * D),  # 4 q heads
                    np.arange(q_dim + g * D, q_dim + (g + 1) * D),  # k head
                    np.arange(q_dim + kv_dim + g * D, q_dim + kv_dim + (g + 1) * D),
                ]
            )
            wq = W_qkv[:, cols].astype(BF16)  # [C, 768]
            wq = np.ascontiguousarray(
                wq.reshape(CC, 128, NF * 128).transpose(1, 0, 2)
            )  # [128, CC, 768]
            bq = np.ascontiguousarray(
                b_qkv[cols].astype(np.float32).reshape(NF, 128).T
            )  # [128, NF]
            wo = W_out[4 * g * D : (4 * g + 4) * D, :].astype(BF16)  # [512, C]
            wo = np.ascontiguousarray(
                wo.reshape(GQ, 128, C).transpose(1, 0, 2)
            )  # [128, GQ, C]
            in_maps.append(
                {
                    "xt": xT,
                    "wqkv": wq,
                    "bqkv": bq,
                    "cosT": cos2T,
                    "sinT": sinsT,
                    "swp": swap,
                    "idn": iden,
                    "wout": wo,
                }
            )
    return in_maps


def kernel(x, rope_cache, W_qkv, b_qkv, W_out, b_out, _trace=False):
    from concourse.bass_utils import run_bass_kernel_spmd

    if "nc" not in _CACHED:
        _CACHED["nc"] = _build_bass()
    nc = _CACHED["nc"]

    in_maps = _host_prep(
        np.asarray(x), np.asarray(rope_cache), np.asarray(W_qkv),
        np.asarray(b_qkv), np.asarray(W_out),
    )
    res = run_bass_kernel_spmd(nc, in_maps, core_ids=list(range(N_CORES)), trace=_trace)
    _CACHED["last_result"] = res

    out = np.zeros((B, T, C), np.float32)
    for b in range(B):
        acc = res.results[b * NKV]["out"].astype(np.float32)
        for g in range(1, NKV):
            acc = acc + res.results[b * NKV + g]["out"]
        out[b] = acc + np.asarray(b_out)[None, :]
    return out



# revision 9
# speedup vs baseline: 1.1833x; 1.1833x over previous
"""GQA causal self-attention (B=2, T=2048, C=2048, 16 Q heads / 4 KV heads,
head_dim=128) on 8 TRN2 NeuronCores.

Sharding: core = (batch b, kv-group g) for b in {0,1}, g in {0..3}.
Each core computes its batch's 4 Q heads that share KV head g, plus the
partial out-projection over those heads' rows of W_out. Host sums the 4
partials per batch and adds b_out.

v2 layout/engine choices:
  - q/k projection in fp8e4 DoubleRow (weights pre-scaled x64 on host,
    un-scaled in the ACT PSUM-drain which also adds the bias); v stays bf16.
  - v [d,t] -> [t,d] transposes via DMA-transpose (no PE).
  - attention jt-outer: per key-tile, all 4 heads share the k/v stationary
    operand; av matmuls software-pipelined 2 tiles behind the scores so PE
    never waits for ACT's exp.
  - softmax denominator off PE: DVE accumulates P tiles (bf16), gpsimd
    partition_all_reduce gives the broadcast row-sum, DVE fast-reciprocal
    and multiply produce normalized y^T.
  - out-projection for slice s interleaved into slice s+1's first two key
    tiles; PSUM: scores(2) + psy(4) + outproj/swap(2) banks.
"""

import sys

if "/opt/trn_rl_repo" not in sys.path:
    sys.path.insert(0, "/opt/trn_rl_repo")

import numpy as np
import ml_dtypes

BF16 = ml_dtypes.bfloat16

B = 2
T = 2048
C = 2048
NH = 16
NKV = 4
D = 128
GQ = NH // NKV  # 4 q heads per kv head
N_CORES = 8
CC = C // 128  # 16 contraction chunks
TS = T // 512  # 4 t-slices
TT = T // 128  # 16 t-tiles
NQK = GQ + 1  # fp8 feature chunks per core: 4 q heads + k
WSCALE = 64.0  # fp8 weight pre-scale

_CACHED = {}


def _build_bass(reps=1):
    import concourse.bass as bass
    import concourse.bacc as bacc
    import concourse.tile as tile
    import concourse.mybir as mybir

    bf = mybir.dt.bfloat16
    f32 = mybir.dt.float32
    f8 = mybir.dt.float8e4

    nc = bacc.Bacc(None, target_bir_lowering=False)

    xt8_d = nc.dram_tensor("xt8", [128, CC, T], f8, kind="ExternalInput")
    xtb_d = nc.dram_tensor("xtb", [128, CC, T], bf, kind="ExternalInput")
    wqk8_d = nc.dram_tensor("wqk8", [128, CC, NQK * 128], f8, kind="ExternalInput")
    wv_d = nc.dram_tensor("wv", [128, CC, 128], bf, kind="ExternalInput")
    bqkv_d = nc.dram_tensor("bqkv", [128, NQK + 1], f32, kind="ExternalInput")
    cos_d = nc.dram_tensor("cosT", [128, T], bf, kind="ExternalInput")
    sin_d = nc.dram_tensor("sinT", [128, T], bf, kind="ExternalInput")
    swap_d = nc.dram_tensor("swp", [128, 128], bf, kind="ExternalInput")
    wout_d = nc.dram_tensor("wout", [128, GQ, C], bf, kind="ExternalInput")
    out_d = nc.dram_tensor("out", [T, C], f32, kind="ExternalOutput")

    with tile.TileContext(nc) as tc:
        with (
            tc.tile_pool(name="persist", bufs=1) as pers,
            tc.tile_pool(name="xt", bufs=2) as xtp,
            tc.tile_pool(name="stage", bufs=4) as stg,
            tc.tile_pool(name="ptile", bufs=6) as ptp,
            tc.tile_pool(name="pacc", bufs=2) as pap,
            tc.tile_pool(name="small", bufs=2) as smp,
            tc.tile_pool(name="osb", bufs=3) as osp,
            tc.tile_pool(name="ps_a", bufs=4, space="PSUM") as ppa,
            tc.tile_pool(name="ps_y", bufs=1, space="PSUM") as ppy,
            tc.tile_pool(name="ps_o", bufs=2, space="PSUM") as ppo,
        ):
            import contextlib
            loop_cm = tc.For_i(0, reps, 1) if reps > 1 else contextlib.nullcontext()
            with loop_cm:
                _body(nc, tc, mybir, bf, f32, f8,
                      pers, xtp, stg, ptp, pap, smp, osp, ppa, ppy, ppo,
                      xt8_d, xtb_d, wqk8_d, wv_d, bqkv_d, cos_d, sin_d, swap_d,
                      wout_d, out_d)
    nc.compile()
    return nc


def _body(nc, tc, mybir, bf, f32, f8,
          pers, xtp, stg, ptp, pap, smp, osp, ppa, ppy, ppo,
          xt8_d, xtb_d, wqk8_d, wv_d, bqkv_d, cos_d, sin_d, swap_d,
          wout_d, out_d):
    Exp = mybir.ActivationFunctionType.Exp
    Ident = mybir.ActivationFunctionType.Identity
    DR = mybir.MatmulPerfMode.DoubleRow
    mul = mybir.AluOpType.mult
    add = mybir.AluOpType.add
    import concourse.bass_isa as bass_isa

    # ---- persistent loads, in need-order ----
    wqk_sb = pers.tile([128, CC, NQK * 128], f8)
    xt8_0 = xtp.tile([128, CC, 512], f8, tag="xt8")
    # interleave weight/x chunk DMAs so the first matmul can start early
    for c2 in range(CC // 2):
        nc.sync.dma_start(wqk_sb[:, 2 * c2 : 2 * c2 + 2, :],
                          wqk8_d[:, 2 * c2 : 2 * c2 + 2, :])
        nc.sync.dma_start(xt8_0[:, 2 * c2 : 2 * c2 + 2, :],
                          xt8_d[:, 2 * c2 : 2 * c2 + 2, 0:512])
    bq_sb = pers.tile([128, NQK + 1], f32)
    nc.sync.dma_start(bq_sb[:], bqkv_d[:])
    swap_sb = pers.tile([128, 128], bf)
    nc.sync.dma_start(swap_sb[:], swap_d[:])
    cos_sb = pers.tile([128, T], bf)
    nc.sync.dma_start(cos_sb[:], cos_d[:])
    sin_sb = pers.tile([128, T], bf)
    nc.sync.dma_start(sin_sb[:], sin_d[:])
    wv_sb = pers.tile([128, CC, 128], bf)
    nc.sync.dma_start(wv_sb[:], wv_d[:])
    xtb_0 = xtp.tile([128, CC, 512], bf, tag="xtb")
    nc.sync.dma_start(xtb_0[:], xtb_d[:, :, 0:512])
    wout_sb = pers.tile([128, GQ, C], bf)
    nc.sync.dma_start(wout_sb[:], wout_d[:])

    # persistent activations
    qk_sb = pers.tile([128, NQK, T], bf)  # rotated q0..q3, k
    v_sb = pers.tile([128, TT, 128], bf)  # v in [t-part, d] tiles
    y_sb = pers.tile([128, GQ, T], bf)  # y^T per head

    # ---- phase 1: QKV + RoPE + v transpose ----
    rope_q = []  # deferred swap-matmul chains: (raw, f, tsl)

    def emit_rope(raw, f, tsl):
        psw = ppo.tile([128, 512], f32, tag="ops")
        nc.tensor.matmul(psw[:], swap_sb[:], raw[:], start=True, stop=True)
        tmp = stg.tile([128, 512], bf, tag="ropetmp")
        nc.vector.tensor_tensor(tmp[:], psw[:], sin_sb[:, tsl], mul)
        nc.vector.tensor_tensor(qk_sb[:, f, tsl], raw[:], cos_sb[:, tsl], mul)
        nc.vector.tensor_tensor(qk_sb[:, f, tsl], qk_sb[:, f, tsl], tmp[:], add)

    for ts in range(TS):
        tsl = slice(ts * 512, (ts + 1) * 512)
        if ts == 0:
            xt8 = xt8_0
            xtb = xtb_0
        else:
            xt8 = xtp.tile([128, CC, 512], f8, tag="xt8")
            nc.sync.dma_start(xt8[:], xt8_d[:, :, tsl])
            xtb = xtp.tile([128, CC, 512], bf, tag="xtb")
            nc.sync.dma_start(xtb[:], xtb_d[:, :, tsl])
        for f in range(NQK):
            ps = ppa.tile([128, 512], f32, tag="aps")
            for c2 in range(CC // 2):
                nc.tensor.matmul(
                    ps[:],
                    wqk_sb[:, 2 * c2 : 2 * c2 + 2, f * 128 : (f + 1) * 128],
                    xt8[:, 2 * c2 : 2 * c2 + 2, :],
                    start=(c2 == 0),
                    stop=(c2 == CC // 2 - 1),
                    perf_mode=DR,
                )
            # PSUM drain + un-scale + bias, bf16 out
            raw = stg.tile([128, 512], bf, tag="raw")
            nc.scalar.activation(
                raw[:], ps[:], Ident,
                bias=bq_sb[:, f : f + 1], scale=1.0 / WSCALE,
            )
            if rope_q:
                emit_rope(*rope_q.pop(0))
            rope_q.append((raw, f, tsl))
        # v: bf16 matmul
        ps = ppa.tile([128, 512], f32, tag="aps")
        for cc in range(CC):
            nc.tensor.matmul(
                ps[:],
                wv_sb[:, cc, :],
                xtb[:, cc, :],
                start=(cc == 0),
                stop=(cc == CC - 1),
            )
        vraw = stg.tile([128, 512], bf, tag="raw")
        nc.scalar.activation(
            vraw[:], ps[:], Ident, bias=bq_sb[:, NQK : NQK + 1], scale=1.0
        )
        if rope_q:
            emit_rope(*rope_q.pop(0))
        # v transpose [d, t] -> [t, d] via DMA xbar
        for k in range(4):
            nc.sync.dma_start_transpose(
                v_sb[:, ts * 4 + k, :], vraw[:, k * 128 : (k + 1) * 128]
            )
    while rope_q:
        emit_rope(*rope_q.pop(0))

    # ---- phase 2+3: attention (h-outer, pipelined av) with the previous
    # slice's out-projection sprinkled one PSUM-group per iteration ----
    PIPE = 3  # av/den lag this many score tiles behind

    def outproj_group(tt, es):
        pso = ppo.tile([128, 512], f32, tag="ops")
        for h in range(GQ):
            nc.tensor.matmul(
                pso[:],
                y_sb[:, h, tt * 128 : (tt + 1) * 128],
                wout_sb[:, h, es * 512 : (es + 1) * 512],
                start=(h == 0),
                stop=(h == GQ - 1),
            )
        o_sb = osb_cur[0]
        osl = slice(es * 512, (es + 1) * 512)
        if es % 2 == 0:
            nc.vector.tensor_copy(o_sb[:, osl], pso[:])
        else:
            nc.scalar.copy(o_sb[:, osl], pso[:])
        if es == 3:
            nc.sync.dma_start(out_d[tt * 128 : (tt + 1) * 128, :], o_sb[:])

    osb_cur = [None]

    def next_outproj_work(s):
        # yields (tt, es) pairs for slice s's out-projection
        for tt in range(4 * s, 4 * s + 4):
            osb_cur[0] = osp.tile([128, C], f32, tag="osb", name="osb")
            for es in range(4):
                yield tt, es

    for s in range(TS):
        isl = slice(s * 512, (s + 1) * 512)
        njt = 4 * (s + 1)
        pacc = [
            pap.tile([128, 512], bf, tag=f"pacc{h}", name=f"pacc{h}")
            for h in range(GQ)
        ]
        op_iter = iter(next_outproj_work(s - 1)) if s > 0 else None
        it = 0  # iteration count within this slice

        for h in range(GQ):
            psy = ppy.tile([128, 512], f32, tag=f"yps{h % 2}")
            pend = []  # (P tile, jt, off)

            def drain_one():
                P, jt, off = pend.pop(0)
                nc.tensor.matmul(
                    psy[:, off:512],
                    v_sb[:, jt, :],
                    P[:, off:512],
                    start=(jt == 0),
                    stop=(jt == njt - 1),
                )
                if jt == 0:
                    nc.vector.tensor_copy(pacc[h][:], P[:])
                else:
                    nc.vector.tensor_tensor(
                        pacc[h][:, off:512], pacc[h][:, off:512], P[:, off:512], add
                    )

            for jt in range(njt):
                off = max(0, 128 * jt - 512 * s)
                pss = ppa.tile([128, 512], f32, tag="aps")
                nc.tensor.matmul(
                    pss[:, off:512],
                    qk_sb[:, GQ, jt * 128 : (jt + 1) * 128],
                    qk_sb[:, h, s * 512 + off : (s + 1) * 512],
                    start=True,
                    stop=True,
                )
                P = ptp.tile([128, 512], bf, tag="P")
                nc.scalar.activation(
                    P[:, off:512], pss[:, off:512], Exp, scale=1.0 / 128.0
                )
                if jt >= 4 * s:
                    nc.gpsimd.affine_select(
                        out=P[:, off : off + 128],
                        in_=P[:, off : off + 128],
                        pattern=[[1, 128]],
                        compare_op=mybir.AluOpType.is_ge,
                        fill=0.0,
                        base=0,
                        channel_multiplier=-1,
                    )
                while len(pend) >= PIPE:
                    drain_one()
                pend.append((P, jt, off))
                # sprinkle one out-proj PSUM group per iteration (from it=3,
                # giving the previous slice's last y-normalize time to land)
                if op_iter is not None and it >= 3:
                    for tt_es in op_iter:
                        outproj_group(*tt_es)
                        break
                it += 1
            while pend:
                drain_one()

            # normalize: den = partition-sum of pacc, y = psy / den
            rdb = smp.tile([128, 512], f32, tag="rdb")
            nc.gpsimd.partition_all_reduce(
                rdb[:], pacc[h][:], channels=128, reduce_op=bass_isa.ReduceOp.add
            )
            rinv = smp.tile([128, 512], f32, tag="rinv")
            nc.vector.reciprocal_approx_fast(rinv[:], rdb[:])
            nc.vector.tensor_tensor(y_sb[:, h, isl], psy[:], rinv[:], mul)

        if op_iter is not None:
            for tt_es in op_iter:
                outproj_group(*tt_es)

    for tt_es in next_outproj_work(TS - 1):
        outproj_group(*tt_es)


def _host_prep(x, rope_cache, W_qkv, b_qkv, W_out):
    """Build the 8 per-core input dicts."""
    import concourse.mybir as mybir

    F8 = mybir.dt.np(mybir.dt.float8e4)
    q_dim = NH * D  # 2048
    kv_dim = NKV * D  # 512

    # rope tables in [d, t] layout
    sin = rope_cache[:, 0::2].astype(np.float32)  # [T, 64]
    cos = rope_cache[:, 1::2].astype(np.float32)
    cos2T = np.empty((128, T), np.float32)
    sinsT = np.empty((128, T), np.float32)
    cos2T[0::2] = cos.T
    cos2T[1::2] = cos.T
    sinsT[0::2] = -sin.T
    sinsT[1::2] = sin.T
    cos2T = cos2T.astype(BF16)
    sinsT = sinsT.astype(BF16)

    swap = np.zeros((128, 128), BF16)
    idx = np.arange(128)
    swap[idx, idx ^ 1] = 1

    def tile_cols(w, ncols):
        # [C, ncols*128] -> [128, CC, ncols*128] with contraction c = cc*128+p
        return np.ascontiguousarray(
            w.reshape(CC, 128, ncols * 128).transpose(1, 0, 2)
        )

    in_maps = []
    for b in range(B):
        xT = np.ascontiguousarray(x[b].T)  # [C, T] f32
        xT = xT.reshape(CC, 128, T).transpose(1, 0, 2)  # [128, CC, T]
        xt8 = np.ascontiguousarray(np.clip(xT, -240, 240).astype(F8))
        xtb = np.ascontiguousarray(xT.astype(BF16))
        for g in range(NKV):
            qk_cols = np.concatenate(
                [
                    np.arange(4 * g * D, (4 * g + 4) * D),  # 4 q heads
                    np.arange(q_dim + g * D, q_dim + (g + 1) * D),  # k head
                ]
            )
            v_cols = np.arange(q_dim + kv_dim + g * D, q_dim + kv_dim + (g + 1) * D)
            wqk = np.clip(W_qkv[:, qk_cols] * WSCALE, -240, 240).astype(F8)
            wv = W_qkv[:, v_cols].astype(BF16)
            bq = np.ascontiguousarray(
                b_qkv[np.concatenate([qk_cols, v_cols])]
                .astype(np.float32).reshape(NQK + 1, 128).T
            )  # [128, NQK+1]
            wo = W_out[4 * g * D : (4 * g + 4) * D, :].astype(BF16)  # [512, C]
            wo = np.ascontiguousarray(
                wo.reshape(GQ, 128, C).transpose(1, 0, 2)
            )  # [128, GQ, C]
            in_maps.append(
                {
                    "xt8": xt8,
                    "xtb": xtb,
                    "wqk8": tile_cols(wqk, NQK),
                    "wv": tile_cols(wv, 1),
                    "bqkv": bq,
                    "cosT": cos2T,
                    "sinT": sinsT,
                    "swp": swap,
                    "wout": wo,
                }
            )
    return in_maps


def kernel(x, rope_cache, W_qkv, b_qkv, W_out, b_out, _trace=False):
    from concourse.bass_utils import run_bass_kernel_spmd

    if "nc" not in _CACHED:
        _CACHED["nc"] = _build_bass()
    nc = _CACHED["nc"]

    in_maps = _host_prep(
        np.asarray(x), np.asarray(rope_cache), np.asarray(W_qkv),
        np.asarray(b_qkv), np.asarray(W_out),
    )
    res = run_bass_kernel_spmd(nc, in_maps, core_ids=list(range(N_CORES)), trace=_trace)
    _CACHED["last_result"] = res

    out = np.zeros((B, T, C), np.float32)
    for b in range(B):
        acc = res.results[b * NKV]["out"].astype(np.float32)
        for g in range(1, NKV):
            acc = acc + res.results[b * NKV + g]["out"]
        out[b] = acc + np.asarray(b_out)[None, :]
    return out


# revision 19
# speedup vs baseline: 1.2706x; 1.0738x over previous
"""GQA causal self-attention (B=2, T=2048, C=2048, 16 Q heads / 4 KV heads,
head_dim=128) on 8 TRN2 NeuronCores.

Sharding: core = (batch b, kv-group g) for b in {0,1}, g in {0..3}.
Each core computes its batch's 4 Q heads that share KV head g, plus the
partial out-projection over those heads' rows of W_out. Host sums the 4
partials per batch and adds b_out.

v2 layout/engine choices:
  - q/k projection in fp8e4 DoubleRow (weights pre-scaled x64 on host,
    un-scaled in the ACT PSUM-drain which also adds the bias); v stays bf16.
  - v [d,t] -> [t,d] transposes via DMA-transpose (no PE).
  - attention jt-outer: per key-tile, all 4 heads share the k/v stationary
    operand; av matmuls software-pipelined 2 tiles behind the scores so PE
    never waits for ACT's exp.
  - softmax denominator off PE: DVE accumulates P tiles (bf16), gpsimd
    partition_all_reduce gives the broadcast row-sum, DVE fast-reciprocal
    and multiply produce normalized y^T.
  - out-projection for slice s interleaved into slice s+1's first two key
    tiles; PSUM: scores(2) + psy(4) + outproj/swap(2) banks.
"""

import sys

if "/opt/trn_rl_repo" not in sys.path:
    sys.path.insert(0, "/opt/trn_rl_repo")

import numpy as np
import ml_dtypes

BF16 = ml_dtypes.bfloat16

B = 2
T = 2048
C = 2048
NH = 16
NKV = 4
D = 128
GQ = NH // NKV  # 4 q heads per kv head
N_CORES = 8
CC = C // 128  # 16 contraction chunks
TS = T // 512  # 4 t-slices
TT = T // 128  # 16 t-tiles
NQK = GQ + 1  # fp8 feature chunks per core: 4 q heads + k
WSCALE = 64.0  # fp8 weight pre-scale

_CACHED = {}


def _build_bass(reps=1):
    import concourse.bass as bass
    import concourse.bacc as bacc
    import concourse.tile as tile
    import concourse.mybir as mybir

    bf = mybir.dt.bfloat16
    f32 = mybir.dt.float32
    f8 = mybir.dt.float8e4

    nc = bacc.Bacc(None, target_bir_lowering=False)

    xt8_d = nc.dram_tensor("xt8", [128, TS, CC, 512], f8, kind="ExternalInput")
    xtb_d = nc.dram_tensor("xtb", [128, TS, CC, 512], bf, kind="ExternalInput")
    wqk8_d = nc.dram_tensor("wqk8", [128, CC, NQK * 128], f8, kind="ExternalInput")
    wv_d = nc.dram_tensor("wv", [128, CC, 128], bf, kind="ExternalInput")
    bqkv_d = nc.dram_tensor("bqkv", [128, NQK + 1], f32, kind="ExternalInput")
    cos_d = nc.dram_tensor("cosT", [128, T], bf, kind="ExternalInput")
    sin_d = nc.dram_tensor("sinT", [128, T], bf, kind="ExternalInput")
    swap_d = nc.dram_tensor("swp", [128, 128], bf, kind="ExternalInput")
    wout_d = nc.dram_tensor("wout", [128, GQ, C], bf, kind="ExternalInput")
    out_d = nc.dram_tensor("out", [T, C], f32, kind="ExternalOutput")

    with tile.TileContext(nc) as tc:
        with (
            tc.tile_pool(name="persist", bufs=1) as pers,
            tc.tile_pool(name="xt", bufs=2) as xtp,
            tc.tile_pool(name="stage", bufs=4) as stg,
            tc.tile_pool(name="ptile", bufs=6) as ptp,
            tc.tile_pool(name="pacc", bufs=2) as pap,
            tc.tile_pool(name="small", bufs=2) as smp,
            tc.tile_pool(name="osb", bufs=3) as osp,
            tc.tile_pool(name="ps_a", bufs=3, space="PSUM") as ppa,
            tc.tile_pool(name="ps_y", bufs=1, space="PSUM") as ppy,
            tc.tile_pool(name="ps_o", bufs=3, space="PSUM") as ppo,
        ):
            import contextlib
            loop_cm = tc.For_i(0, reps, 1) if reps > 1 else contextlib.nullcontext()
            with loop_cm:
                _body(nc, tc, mybir, bf, f32, f8,
                      pers, xtp, stg, ptp, pap, smp, osp, ppa, ppy, ppo,
                      xt8_d, xtb_d, wqk8_d, wv_d, bqkv_d, cos_d, sin_d, swap_d,
                      wout_d, out_d)
    nc.compile()
    return nc


def _body(nc, tc, mybir, bf, f32, f8,
          pers, xtp, stg, ptp, pap, smp, osp, ppa, ppy, ppo,
          xt8_d, xtb_d, wqk8_d, wv_d, bqkv_d, cos_d, sin_d, swap_d,
          wout_d, out_d):
    Exp = mybir.ActivationFunctionType.Exp
    Ident = mybir.ActivationFunctionType.Identity
    DR = mybir.MatmulPerfMode.DoubleRow
    mul = mybir.AluOpType.mult
    add = mybir.AluOpType.add

    # ---- persistent loads, in need-order ----
    wqk_sb = pers.tile([128, CC, NQK * 128], f8)
    xt8_0 = xtp.tile([128, CC, 512], f8, tag="xt8")
    # interleave weight/x quarter DMAs so the first matmul can start early
    for c4 in range(CC // 4):
        q = slice(4 * c4, 4 * c4 + 4)
        nc.sync.dma_start(wqk_sb[:, q, :], wqk8_d[:, q, :])
        nc.sync.dma_start(xt8_0[:, q, :], xt8_d[:, 0, q, :])
    bq_sb = pers.tile([128, NQK + 1], f32)
    nc.sync.dma_start(bq_sb[:], bqkv_d[:])
    swap_sb = pers.tile([128, 128], bf)
    nc.sync.dma_start(swap_sb[:], swap_d[:])
    cos_sb = pers.tile([128, T], bf)
    nc.sync.dma_start(cos_sb[:], cos_d[:])
    sin_sb = pers.tile([128, T], bf)
    nc.sync.dma_start(sin_sb[:], sin_d[:])
    wv_sb = pers.tile([128, CC, 128], bf)
    nc.sync.dma_start(wv_sb[:], wv_d[:])
    xtb_0 = xtp.tile([128, CC, 512], bf, tag="xtb")
    nc.sync.dma_start(xtb_0[:], xtb_d[:, 0])
    wout_sb = pers.tile([128, GQ, C], bf)
    nc.sync.dma_start(wout_sb[:], wout_d[:])
    ones_sb = pers.tile([128, 128], bf)
    nc.vector.memset(ones_sb[:], 1.0)

    # persistent activations
    qk_sb = pers.tile([128, NQK, T], bf)  # rotated q0..q3, k
    v_sb = pers.tile([128, TT, 128], bf)  # v in [t-part, d] tiles
    y_sb = pers.tile([128, GQ, T], bf)  # y^T per head

    # ---- phase 1: QKV + RoPE + v transpose ----
    rope_q = []  # deferred swap-matmul chains: (raw, f, tsl)

    def emit_rope(raw, f, tsl):
        psw = ppo.tile([128, 512], f32, tag="ops")
        nc.tensor.matmul(psw[:], swap_sb[:], raw[:], start=True, stop=True)
        tmp = stg.tile([128, 512], bf, tag="ropetmp")
        nc.vector.tensor_tensor(tmp[:], psw[:], sin_sb[:, tsl], mul)
        nc.vector.tensor_tensor(qk_sb[:, f, tsl], raw[:], cos_sb[:, tsl], mul)
        nc.vector.tensor_tensor(qk_sb[:, f, tsl], qk_sb[:, f, tsl], tmp[:], add)

    for ts in range(TS):
        tsl = slice(ts * 512, (ts + 1) * 512)
        if ts == 0:
            xt8 = xt8_0
            xtb = xtb_0
        else:
            xt8 = xtp.tile([128, CC, 512], f8, tag="xt8")
            nc.sync.dma_start(xt8[:], xt8_d[:, ts])
            xtb = xtp.tile([128, CC, 512], bf, tag="xtb")
            nc.sync.dma_start(xtb[:], xtb_d[:, ts])
        for f in range(NQK):
            ps = ppa.tile([128, 512], f32, tag="aps")
            for c2 in range(CC // 2):
                nc.tensor.matmul(
                    ps[:],
                    wqk_sb[:, 2 * c2 : 2 * c2 + 2, f * 128 : (f + 1) * 128],
                    xt8[:, 2 * c2 : 2 * c2 + 2, :],
                    start=(c2 == 0),
                    stop=(c2 == CC // 2 - 1),
                    perf_mode=DR,
                )
            # PSUM drain + un-scale + bias, bf16 out
            raw = stg.tile([128, 512], bf, tag="raw")
            nc.scalar.activation(
                raw[:], ps[:], Ident,
                bias=bq_sb[:, f : f + 1], scale=1.0 / WSCALE,
            )
            if rope_q:
                emit_rope(*rope_q.pop(0))
            rope_q.append((raw, f, tsl))
        # v: bf16 matmul
        ps = ppa.tile([128, 512], f32, tag="aps")
        for cc in range(CC):
            nc.tensor.matmul(
                ps[:],
                wv_sb[:, cc, :],
                xtb[:, cc, :],
                start=(cc == 0),
                stop=(cc == CC - 1),
            )
        vraw = stg.tile([128, 512], bf, tag="raw")
        nc.scalar.activation(
            vraw[:], ps[:], Ident, bias=bq_sb[:, NQK : NQK + 1], scale=1.0
        )
        if rope_q:
            emit_rope(*rope_q.pop(0))
        # v transpose [d, t] -> [t, d] via DMA xbar
        for k in range(4):
            nc.sync.dma_start_transpose(
                v_sb[:, ts * 4 + k, :], vraw[:, k * 128 : (k + 1) * 128]
            )
    while rope_q:
        emit_rope(*rope_q.pop(0))

    # ---- phase 2+3: attention (h-outer, pipelined av) with the previous
    # slice's out-projection sprinkled one PSUM-group per iteration ----
    PIPE = 3  # av/den lag this many score tiles behind

    def outproj_group(tt, es):
        pso = ppo.tile([128, 512], f32, tag="ops")
        for h in range(GQ):
            nc.tensor.matmul(
                pso[:],
                y_sb[:, h, tt * 128 : (tt + 1) * 128],
                wout_sb[:, h, es * 512 : (es + 1) * 512],
                start=(h == 0),
                stop=(h == GQ - 1),
            )
        o_sb = osb_cur[0]
        osl = slice(es * 512, (es + 1) * 512)
        if es % 2 == 0:
            nc.vector.tensor_copy(o_sb[:, osl], pso[:])
        else:
            nc.scalar.copy(o_sb[:, osl], pso[:])
        if es == 3:
            nc.sync.dma_start(out_d[tt * 128 : (tt + 1) * 128, :], o_sb[:])

    osb_cur = [None]

    def next_outproj_work(s):
        # yields (tt, es) pairs for slice s's out-projection
        for tt in range(4 * s, 4 * s + 4):
            osb_cur[0] = osp.tile([128, C], f32, tag="osb", name="osb")
            for es in range(4):
                yield tt, es

    for s in range(TS):
        isl = slice(s * 512, (s + 1) * 512)
        njt = 4 * (s + 1)
        pacc = [
            pap.tile([128, 512], bf, tag=f"pacc{h}", name=f"pacc{h}")
            for h in range(GQ)
        ]
        op_iter = iter(next_outproj_work(s - 1)) if s > 0 else None
        it = 0  # iteration count within this slice
        # spread the 16 out-proj groups across this slice's iterations
        op_stride = max(1, (16 * (s + 1) - 6) // 16)

        for h in range(GQ):
            psy = ppy.tile([128, 512], f32, tag=f"yps{h % 2}")
            pend = []  # (P tile, jt, off)

            acc_eng = nc.vector if h % 2 == 0 else nc.gpsimd

            def drain_one():
                P, jt, off = pend.pop(0)
                nc.tensor.matmul(
                    psy[:, off:512],
                    v_sb[:, jt, :],
                    P[:, off:512],
                    start=(jt == 0),
                    stop=(jt == njt - 1),
                )
                if jt == 0:
                    acc_eng.tensor_copy(pacc[h][:], P[:])
                else:
                    acc_eng.tensor_tensor(
                        pacc[h][:, off:512], pacc[h][:, off:512], P[:, off:512], add
                    )

            for jt in range(njt):
                off = max(0, 128 * jt - 512 * s)
                pss = ppa.tile([128, 512], f32, tag="aps")
                nc.tensor.matmul(
                    pss[:, off:512],
                    qk_sb[:, GQ, jt * 128 : (jt + 1) * 128],
                    qk_sb[:, h, s * 512 + off : (s + 1) * 512],
                    start=True,
                    stop=True,
                )
                P = ptp.tile([128, 512], bf, tag="P")
                nc.scalar.activation(
                    P[:, off:512], pss[:, off:512], Exp, scale=1.0 / 128.0
                )
                if jt >= 4 * s:
                    nc.gpsimd.affine_select(
                        out=P[:, off : off + 128],
                        in_=P[:, off : off + 128],
                        pattern=[[1, 128]],
                        compare_op=mybir.AluOpType.is_ge,
                        fill=0.0,
                        base=0,
                        channel_multiplier=-1,
                    )
                while len(pend) >= PIPE:
                    drain_one()
                pend.append((P, jt, off))
                # sprinkle out-proj PSUM groups across the slice (from it=3,
                # giving the previous slice's last y-normalize time to land)
                if op_iter is not None and it >= 3 and (it - 3) % op_stride == 0:
                    for tt_es in op_iter:
                        outproj_group(*tt_es)
                        break
                it += 1
            while pend:
                drain_one()

            # normalize: den broadcast over partitions via ones-matmul,
            # then y = psy * (1/den)
            psd = ppo.tile([128, 512], f32, tag="ops")
            nc.tensor.matmul(psd[:], ones_sb[:], pacc[h][:], start=True, stop=True)
            rinv = smp.tile([128, 512], f32, tag="rinv")
            nc.vector.reciprocal_approx_fast(rinv[:], psd[:])
            nc.vector.tensor_tensor(y_sb[:, h, isl], psy[:], rinv[:], mul)

        if op_iter is not None:
            for tt_es in op_iter:
                outproj_group(*tt_es)

    for tt_es in next_outproj_work(TS - 1):
        outproj_group(*tt_es)


def _host_prep(x, rope_cache, W_qkv, b_qkv, W_out):
    """Build the 8 per-core input dicts."""
    import concourse.mybir as mybir

    F8 = mybir.dt.np(mybir.dt.float8e4)
    q_dim = NH * D  # 2048
    kv_dim = NKV * D  # 512

    # rope tables in [d, t] layout
    sin = rope_cache[:, 0::2].astype(np.float32)  # [T, 64]
    cos = rope_cache[:, 1::2].astype(np.float32)
    cos2T = np.empty((128, T), np.float32)
    sinsT = np.empty((128, T), np.float32)
    cos2T[0::2] = cos.T
    cos2T[1::2] = cos.T
    sinsT[0::2] = -sin.T
    sinsT[1::2] = sin.T
    cos2T = cos2T.astype(BF16)
    sinsT = sinsT.astype(BF16)

    swap = np.zeros((128, 128), BF16)
    idx = np.arange(128)
    swap[idx, idx ^ 1] = 1

    def tile_cols(w, ncols):
        # [C, ncols*128] -> [128, CC, ncols*128] with contraction c = cc*128+p
        return np.ascontiguousarray(
            w.reshape(CC, 128, ncols * 128).transpose(1, 0, 2)
        )

    in_maps = []
    for b in range(B):
        xT = np.ascontiguousarray(x[b].T)  # [C, T] f32
        xT = xT.reshape(CC, 128, T).transpose(1, 0, 2)  # [128, CC, T]
        # -> [128, TS, CC, 512]: per-slice DMA reads contiguous lines
        xT = xT.reshape(128, CC, TS, 512).transpose(0, 2, 1, 3)
        xt8 = np.ascontiguousarray(np.clip(xT, -240, 240).astype(F8))
        xtb = np.ascontiguousarray(xT.astype(BF16))
        for g in range(NKV):
            qk_cols = np.concatenate(
                [
                    np.arange(4 * g * D, (4 * g + 4) * D),  # 4 q heads
                    np.arange(q_dim + g * D, q_dim + (g + 1) * D),  # k head
                ]
            )
            v_cols = np.arange(q_dim + kv_dim + g * D, q_dim + kv_dim + (g + 1) * D)
            wqk = np.clip(W_qkv[:, qk_cols] * WSCALE, -240, 240).astype(F8)
            wv = W_qkv[:, v_cols].astype(BF16)
            bq = np.ascontiguousarray(
                b_qkv[np.concatenate([qk_cols, v_cols])]
                .astype(np.float32).reshape(NQK + 1, 128).T
            )  # [128, NQK+1]
            wo = W_out[4 * g * D : (4 * g + 4) * D, :].astype(BF16)  # [512, C]
            wo = np.ascontiguousarray(
                wo.reshape(GQ, 128, C).transpose(1, 0, 2)
            )  # [128, GQ, C]
            in_maps.append(
                {
                    "xt8": xt8,
                    "xtb": xtb,
                    "wqk8": tile_cols(wqk, NQK),
                    "wv": tile_cols(wv, 1),
                    "bqkv": bq,
                    "cosT": cos2T,
                    "sinT": sinsT,
                    "swp": swap,
                    "wout": wo,
                }
            )
    return in_maps


def kernel(x, rope_cache, W_qkv, b_qkv, W_out, b_out, _trace=False):
    from concourse.bass_utils import run_bass_kernel_spmd

    if "nc" not in _CACHED:
        _CACHED["nc"] = _build_bass()
    nc = _CACHED["nc"]

    in_maps = _host_prep(
        np.asarray(x), np.asarray(rope_cache), np.asarray(W_qkv),
        np.asarray(b_qkv), np.asarray(W_out),
    )
    res = run_bass_kernel_spmd(nc, in_maps, core_ids=list(range(N_CORES)), trace=_trace)
    _CACHED["last_result"] = res

    out = np.zeros((B, T, C), np.float32)
    for b in range(B):
        acc = res.results[b * NKV]["out"].astype(np.float32)
        for g in range(1, NKV):
            acc = acc + res.results[b * NKV + g]["out"]
        out[b] = acc + np.asarray(b_out)[None, :]
    return out


# revision 21
# speedup vs baseline: 1.5603x; 1.2280x over previous
"""GQA causal self-attention (B=2, T=2048, C=2048, 16 Q heads / 4 KV heads,
head_dim=128) on 8 TRN2 NeuronCores.

Sharding: core = (batch b, kv-group g) for b in {0,1}, g in {0..3}.
Each core computes its batch's 4 Q heads that share KV head g, plus the
partial out-projection over those heads' rows of W_out. Host sums the 4
partials per batch and adds b_out.

v2 layout/engine choices:
  - q/k projection in fp8e4 DoubleRow (weights pre-scaled x64 on host,
    un-scaled in the ACT PSUM-drain which also adds the bias); v stays bf16.
  - v [d,t] -> [t,d] transposes via DMA-transpose (no PE).
  - attention jt-outer: per key-tile, all 4 heads share the k/v stationary
    operand; av matmuls software-pipelined 2 tiles behind the scores so PE
    never waits for ACT's exp.
  - softmax denominator off PE: DVE accumulates P tiles (bf16), gpsimd
    partition_all_reduce gives the broadcast row-sum, DVE fast-reciprocal
    and multiply produce normalized y^T.
  - out-projection for slice s interleaved into slice s+1's first two key
    tiles; PSUM: scores(2) + psy(4) + outproj/swap(2) banks.
"""

import sys

if "/opt/trn_rl_repo" not in sys.path:
    sys.path.insert(0, "/opt/trn_rl_repo")

import numpy as np
import ml_dtypes

BF16 = ml_dtypes.bfloat16

B = 2
T = 2048
C = 2048
NH = 16
NKV = 4
D = 128
GQ = NH // NKV  # 4 q heads per kv head
N_CORES = 8
CC = C // 128  # 16 contraction chunks
TS = T // 512  # 4 t-slices
TT = T // 128  # 16 t-tiles
NQK = GQ + 1  # fp8 feature chunks per core: 4 q heads + k
WSCALE = 64.0  # fp8 weight pre-scale

_CACHED = {}


def _build_bass(reps=1):
    import concourse.bass as bass
    import concourse.bacc as bacc
    import concourse.tile as tile
    import concourse.mybir as mybir

    bf = mybir.dt.bfloat16
    f32 = mybir.dt.float32
    f8 = mybir.dt.float8e4

    nc = bacc.Bacc(None, target_bir_lowering=False)

    xt8_d = nc.dram_tensor("xt8", [128, TS, CC, 512], f8, kind="ExternalInput")
    xtb_d = nc.dram_tensor("xtb", [128, TS, CC, 512], bf, kind="ExternalInput")
    wqk8_d = nc.dram_tensor("wqk8", [128, CC, NQK * 128], f8, kind="ExternalInput")
    wv_d = nc.dram_tensor("wv", [128, CC, 128], bf, kind="ExternalInput")
    bqkv_d = nc.dram_tensor("bqkv", [128, NQK + 1], f32, kind="ExternalInput")
    cos_d = nc.dram_tensor("cosT", [128, T], bf, kind="ExternalInput")
    sin_d = nc.dram_tensor("sinT", [128, T], bf, kind="ExternalInput")
    swap_d = nc.dram_tensor("swp", [128, 128], bf, kind="ExternalInput")
    wout_d = nc.dram_tensor("wout", [128, GQ, C], bf, kind="ExternalInput")
    out_d = nc.dram_tensor("out", [T, C], f32, kind="ExternalOutput")

    with tile.TileContext(nc) as tc:
        with (
            tc.tile_pool(name="persist", bufs=1) as pers,
            tc.tile_pool(name="xt", bufs=2) as xtp,
            tc.tile_pool(name="stage", bufs=4) as stg,
            tc.tile_pool(name="ptile", bufs=6) as ptp,
            tc.tile_pool(name="pacc", bufs=2) as pap,
            tc.tile_pool(name="small", bufs=2) as smp,
            tc.tile_pool(name="osb", bufs=3) as osp,
            tc.tile_pool(name="ps_a", bufs=3, space="PSUM") as ppa,
            tc.tile_pool(name="ps_y", bufs=1, space="PSUM") as ppy,
            tc.tile_pool(name="ps_o", bufs=3, space="PSUM") as ppo,
        ):
            import contextlib
            loop_cm = tc.For_i(0, reps, 1) if reps > 1 else contextlib.nullcontext()
            with loop_cm:
                _body(nc, tc, mybir, bf, f32, f8,
                      pers, xtp, stg, ptp, pap, smp, osp, ppa, ppy, ppo,
                      xt8_d, xtb_d, wqk8_d, wv_d, bqkv_d, cos_d, sin_d, swap_d,
                      wout_d, out_d)
    nc.compile()
    return nc


def _body(nc, tc, mybir, bf, f32, f8,
          pers, xtp, stg, ptp, pap, smp, osp, ppa, ppy, ppo,
          xt8_d, xtb_d, wqk8_d, wv_d, bqkv_d, cos_d, sin_d, swap_d,
          wout_d, out_d):
    Exp = mybir.ActivationFunctionType.Exp
    Ident = mybir.ActivationFunctionType.Identity
    DR = mybir.MatmulPerfMode.DoubleRow
    mul = mybir.AluOpType.mult
    add = mybir.AluOpType.add

    # ---- persistent loads, in need-order ----
    wqk_sb = pers.tile([128, CC, NQK * 128], f8)
    xt8_0 = xtp.tile([128, CC, 512], f8, tag="xt8")
    # interleave weight/x quarter DMAs so the first matmul can start early
    for c4 in range(CC // 4):
        q = slice(4 * c4, 4 * c4 + 4)
        nc.sync.dma_start(wqk_sb[:, q, :], wqk8_d[:, q, :])
        nc.sync.dma_start(xt8_0[:, q, :], xt8_d[:, 0, q, :])
    bq_sb = pers.tile([128, NQK + 1], f32)
    nc.sync.dma_start(bq_sb[:], bqkv_d[:])
    swap_sb = pers.tile([128, 128], bf)
    nc.sync.dma_start(swap_sb[:], swap_d[:])
    cos_sb = pers.tile([128, T], bf)
    nc.sync.dma_start(cos_sb[:], cos_d[:])
    sin_sb = pers.tile([128, T], bf)
    nc.sync.dma_start(sin_sb[:], sin_d[:])
    wv_sb = pers.tile([128, CC, 128], bf)
    nc.sync.dma_start(wv_sb[:], wv_d[:])
    xtb_0 = xtp.tile([128, CC, 512], bf, tag="xtb")
    nc.sync.dma_start(xtb_0[:], xtb_d[:, 0])
    wout_sb = pers.tile([128, GQ, C], bf)
    nc.sync.dma_start(wout_sb[:], wout_d[:])
    ones_sb = pers.tile([128, 128], bf)
    nc.vector.memset(ones_sb[:], 1.0)

    # persistent activations
    qk_sb = pers.tile([128, NQK, T], bf)  # rotated q0..q3, k
    v_sb = pers.tile([128, TT, 128], bf)  # v in [t-part, d] tiles
    y_sb = pers.tile([128, GQ, T], bf)  # y^T per head

    # ---- phase 1: QKV + RoPE + v transpose ----
    rope_q = []  # deferred swap-matmul chains: (raw, f, tsl)

    def emit_rope(raw, f, tsl):
        psw = ppo.tile([128, 512], f32, tag="ops")
        nc.tensor.matmul(psw[:], swap_sb[:], raw[:], start=True, stop=True)
        tmp = stg.tile([128, 512], bf, tag="ropetmp")
        nc.vector.tensor_tensor(tmp[:], psw[:], sin_sb[:, tsl], mul)
        nc.vector.tensor_tensor(qk_sb[:, f, tsl], raw[:], cos_sb[:, tsl], mul)
        nc.vector.tensor_tensor(qk_sb[:, f, tsl], qk_sb[:, f, tsl], tmp[:], add)

    xt_next = (xt8_0, xtb_0)
    for ts in range(TS):
        tsl = slice(ts * 512, (ts + 1) * 512)
        xt8, xtb = xt_next
        if ts + 1 < TS:
            # prefetch the next slice now, ahead of this slice's v-transposes
            # in the sync queue, so the transfer overlaps this slice's compute
            nxt8 = xtp.tile([128, CC, 512], f8, tag="xt8")
            nc.sync.dma_start(nxt8[:], xt8_d[:, ts + 1])
            nxtb = xtp.tile([128, CC, 512], bf, tag="xtb")
            nc.sync.dma_start(nxtb[:], xtb_d[:, ts + 1])
            xt_next = (nxt8, nxtb)
        for f in range(NQK):
            ps = ppa.tile([128, 512], f32, tag="aps")
            for c2 in range(CC // 2):
                nc.tensor.matmul(
                    ps[:],
                    wqk_sb[:, 2 * c2 : 2 * c2 + 2, f * 128 : (f + 1) * 128],
                    xt8[:, 2 * c2 : 2 * c2 + 2, :],
                    start=(c2 == 0),
                    stop=(c2 == CC // 2 - 1),
                    perf_mode=DR,
                )
            # PSUM drain + un-scale + bias, bf16 out
            raw = stg.tile([128, 512], bf, tag="raw")
            nc.scalar.activation(
                raw[:], ps[:], Ident,
                bias=bq_sb[:, f : f + 1], scale=1.0 / WSCALE,
            )
            if rope_q:
                emit_rope(*rope_q.pop(0))
            rope_q.append((raw, f, tsl))
        # v: bf16 matmul
        ps = ppa.tile([128, 512], f32, tag="aps")
        for cc in range(CC):
            nc.tensor.matmul(
                ps[:],
                wv_sb[:, cc, :],
                xtb[:, cc, :],
                start=(cc == 0),
                stop=(cc == CC - 1),
            )
        vraw = stg.tile([128, 512], bf, tag="raw")
        nc.scalar.activation(
            vraw[:], ps[:], Ident, bias=bq_sb[:, NQK : NQK + 1], scale=1.0
        )
        if rope_q:
            emit_rope(*rope_q.pop(0))
        # v transpose [d, t] -> [t, d] via DMA xbar
        for k in range(4):
            nc.sync.dma_start_transpose(
                v_sb[:, ts * 4 + k, :], vraw[:, k * 128 : (k + 1) * 128]
            )
    while rope_q:
        emit_rope(*rope_q.pop(0))

    # ---- phase 2+3: attention (h-outer, pipelined av) with the previous
    # slice's out-projection sprinkled one PSUM-group per iteration ----
    PIPE = 3  # av/den lag this many score tiles behind

    def outproj_group(tt, es):
        pso = ppo.tile([128, 512], f32, tag="ops")
        for h in range(GQ):
            nc.tensor.matmul(
                pso[:],
                y_sb[:, h, tt * 128 : (tt + 1) * 128],
                wout_sb[:, h, es * 512 : (es + 1) * 512],
                start=(h == 0),
                stop=(h == GQ - 1),
            )
        o_sb = osb_cur[0]
        osl = slice(es * 512, (es + 1) * 512)
        if es % 2 == 0:
            nc.vector.tensor_copy(o_sb[:, osl], pso[:])
        else:
            nc.scalar.copy(o_sb[:, osl], pso[:])
        if es == 3:
            nc.sync.dma_start(out_d[tt * 128 : (tt + 1) * 128, :], o_sb[:])

    osb_cur = [None]

    def next_outproj_work(s):
        # yields (tt, es) pairs for slice s's out-projection
        for tt in range(4 * s, 4 * s + 4):
            osb_cur[0] = osp.tile([128, C], f32, tag="osb", name="osb")
            for es in range(4):
                yield tt, es

    for s in range(TS):
        isl = slice(s * 512, (s + 1) * 512)
        njt = 4 * (s + 1)
        pacc = [
            pap.tile([128, 512], bf, tag=f"pacc{h}", name=f"pacc{h}")
            for h in range(GQ)
        ]
        op_iter = iter(next_outproj_work(s - 1)) if s > 0 else None
        it = 0  # iteration count within this slice
        # spread the 16 out-proj groups across this slice's iterations
        op_stride = max(1, (16 * (s + 1) - 6) // 16)

        for h in range(GQ):
            psy = ppy.tile([128, 512], f32, tag=f"yps{h % 2}")
            pend = []  # (P tile, jt, off)

            acc_eng = nc.vector

            def drain_one():
                P, jt, off = pend.pop(0)
                nc.tensor.matmul(
                    psy[:, off:512],
                    v_sb[:, jt, :],
                    P[:, off:512],
                    start=(jt == 0),
                    stop=(jt == njt - 1),
                )
                if jt == 0:
                    acc_eng.tensor_copy(pacc[h][:], P[:])
                else:
                    acc_eng.tensor_tensor(
                        pacc[h][:, off:512], pacc[h][:, off:512], P[:, off:512], add
                    )

            for jt in range(njt):
                off = max(0, 128 * jt - 512 * s)
                pss = ppa.tile([128, 512], f32, tag="aps")
                nc.tensor.matmul(
                    pss[:, off:512],
                    qk_sb[:, GQ, jt * 128 : (jt + 1) * 128],
                    qk_sb[:, h, s * 512 + off : (s + 1) * 512],
                    start=True,
                    stop=True,
                )
                P = ptp.tile([128, 512], bf, tag="P")
                nc.scalar.activation(
                    P[:, off:512], pss[:, off:512], Exp, scale=1.0 / 128.0
                )
                if jt >= 4 * s:
                    nc.gpsimd.affine_select(
                        out=P[:, off : off + 128],
                        in_=P[:, off : off + 128],
                        pattern=[[1, 128]],
                        compare_op=mybir.AluOpType.is_ge,
                        fill=0.0,
                        base=0,
                        channel_multiplier=-1,
                    )
                while len(pend) >= PIPE:
                    drain_one()
                pend.append((P, jt, off))
                # sprinkle out-proj PSUM groups across the slice (from it=3,
                # giving the previous slice's last y-normalize time to land)
                if op_iter is not None and it >= 3 and (it - 3) % op_stride == 0:
                    for tt_es in op_iter:
                        outproj_group(*tt_es)
                        break
                it += 1
            while pend:
                drain_one()

            # normalize: den broadcast over partitions via ones-matmul,
            # then y = psy * (1/den)
            psd = ppo.tile([128, 512], f32, tag="ops")
            nc.tensor.matmul(psd[:], ones_sb[:], pacc[h][:], start=True, stop=True)
            rinv = smp.tile([128, 512], f32, tag="rinv")
            nc.vector.reciprocal_approx_fast(rinv[:], psd[:])
            nc.vector.tensor_tensor(y_sb[:, h, isl], psy[:], rinv[:], mul)

        if op_iter is not None:
            for tt_es in op_iter:
                outproj_group(*tt_es)

    for tt_es in next_outproj_work(TS - 1):
        outproj_group(*tt_es)


def _host_prep(x, rope_cache, W_qkv, b_qkv, W_out):
    """Build the 8 per-core input dicts."""
    import concourse.mybir as mybir

    F8 = mybir.dt.np(mybir.dt.float8e4)
    q_dim = NH * D  # 2048
    kv_dim = NKV * D  # 512

    # rope tables in [d, t] layout
    sin = rope_cache[:, 0::2].astype(np.float32)  # [T, 64]
    cos = rope_cache[:, 1::2].astype(np.float32)
    cos2T = np.empty((128, T), np.float32)
    sinsT = np.empty((128, T), np.float32)
    cos2T[0::2] = cos.T
    cos2T[1::2] = cos.T
    sinsT[0::2] = -sin.T
    sinsT[1::2] = sin.T
    cos2T = cos2T.astype(BF16)
    sinsT = sinsT.astype(BF16)

    swap = np.zeros((128, 128), BF16)
    idx = np.arange(128)
    swap[idx, idx ^ 1] = 1

    def tile_cols(w, ncols):
        # [C, ncols*128] -> [128, CC, ncols*128] with contraction c = cc*128+p
        return np.ascontiguousarray(
            w.reshape(CC, 128, ncols * 128).transpose(1, 0, 2)
        )

    in_maps = []
    for b in range(B):
        xT = np.ascontiguousarray(x[b].T)  # [C, T] f32
        xT = xT.reshape(CC, 128, T).transpose(1, 0, 2)  # [128, CC, T]
        # -> [128, TS, CC, 512]: per-slice DMA reads contiguous lines
        xT = xT.reshape(128, CC, TS, 512).transpose(0, 2, 1, 3)
        xt8 = np.ascontiguousarray(np.clip(xT, -240, 240).astype(F8))
        xtb = np.ascontiguousarray(xT.astype(BF16))
        for g in range(NKV):
            qk_cols = np.concatenate(
                [
                    np.arange(4 * g * D, (4 * g + 4) * D),  # 4 q heads
                    np.arange(q_dim + g * D, q_dim + (g + 1) * D),  # k head
                ]
            )
            v_cols = np.arange(q_dim + kv_dim + g * D, q_dim + kv_dim + (g + 1) * D)
            wqk = np.clip(W_qkv[:, qk_cols] * WSCALE, -240, 240).astype(F8)
            wv = W_qkv[:, v_cols].astype(BF16)
            bq = np.ascontiguousarray(
                b_qkv[np.concatenate([qk_cols, v_cols])]
                .astype(np.float32).reshape(NQK + 1, 128).T
            )  # [128, NQK+1]
            wo = W_out[4 * g * D : (4 * g + 4) * D, :].astype(BF16)  # [512, C]
            wo = np.ascontiguousarray(
                wo.reshape(GQ, 128, C).transpose(1, 0, 2)
            )  # [128, GQ, C]
            in_maps.append(
                {
                    "xt8": xt8,
                    "xtb": xtb,
                    "wqk8": tile_cols(wqk, NQK),
                    "wv": tile_cols(wv, 1),
                    "bqkv": bq,
                    "cosT": cos2T,
                    "sinT": sinsT,
                    "swp": swap,
                    "wout": wo,
                }
            )
    return in_maps


def kernel(x, rope_cache, W_qkv, b_qkv, W_out, b_out, _trace=False):
    from concourse.bass_utils import run_bass_kernel_spmd

    if "nc" not in _CACHED:
        _CACHED["nc"] = _build_bass()
    nc = _CACHED["nc"]

    in_maps = _host_prep(
        np.asarray(x), np.asarray(rope_cache), np.asarray(W_qkv),
        np.asarray(b_qkv), np.asarray(W_out),
    )
    res = run_bass_kernel_spmd(nc, in_maps, core_ids=list(range(N_CORES)), trace=_trace)
    _CACHED["last_result"] = res

    out = np.zeros((B, T, C), np.float32)
    for b in range(B):
        acc = res.results[b * NKV]["out"].astype(np.float32)
        for g in range(1, NKV):
            acc = acc + res.results[b * NKV + g]["out"]
        out[b] = acc + np.asarray(b_out)[None, :]
    return out


# revision 29
# speedup vs baseline: 1.6445x; 1.0540x over previous
"""GQA causal self-attention (B=2, T=2048, C=2048, 16 Q heads / 4 KV heads,
head_dim=128) on 8 TRN2 NeuronCores.

Sharding: core = (batch b, kv-group g) for b in {0,1}, g in {0..3}.
Each core computes its batch's 4 Q heads that share KV head g, plus the
partial out-projection over those heads' rows of W_out. Host sums the 4
partials per batch and adds b_out.

v2 layout/engine choices:
  - q/k projection in fp8e4 DoubleRow (weights pre-scaled x64 on host,
    un-scaled in the ACT PSUM-drain which also adds the bias); v stays bf16.
  - v [d,t] -> [t,d] transposes via DMA-transpose (no PE).
  - attention jt-outer: per key-tile, all 4 heads share the k/v stationary
    operand; av matmuls software-pipelined 2 tiles behind the scores so PE
    never waits for ACT's exp.
  - softmax denominator off PE: DVE accumulates P tiles (bf16), gpsimd
    partition_all_reduce gives the broadcast row-sum, DVE fast-reciprocal
    and multiply produce normalized y^T.
  - out-projection for slice s interleaved into slice s+1's first two key
    tiles; PSUM: scores(2) + psy(4) + outproj/swap(2) banks.
"""

import sys

if "/opt/trn_rl_repo" not in sys.path:
    sys.path.insert(0, "/opt/trn_rl_repo")

import numpy as np
import ml_dtypes

BF16 = ml_dtypes.bfloat16

B = 2
T = 2048
C = 2048
NH = 16
NKV = 4
D = 128
GQ = NH // NKV  # 4 q heads per kv head
N_CORES = 8
CC = C // 128  # 16 contraction chunks
TS = T // 512  # 4 t-slices
TT = T // 128  # 16 t-tiles
NQK = GQ + 1  # fp8 feature chunks per core: 4 q heads + k
WSCALE = 64.0  # fp8 weight pre-scale

_CACHED = {}


def _build_bass(reps=1):
    import concourse.bass as bass
    import concourse.bacc as bacc
    import concourse.tile as tile
    import concourse.mybir as mybir

    bf = mybir.dt.bfloat16
    f32 = mybir.dt.float32
    f8 = mybir.dt.float8e4

    nc = bacc.Bacc(None, target_bir_lowering=False)

    xt8_d = nc.dram_tensor("xt8", [128, TS, CC, 512], f8, kind="ExternalInput")
    xtb_d = nc.dram_tensor("xtb", [128, TS, CC, 512], bf, kind="ExternalInput")
    wqk8_d = nc.dram_tensor("wqk8", [128, CC, NQK * 128], f8, kind="ExternalInput")
    wv_d = nc.dram_tensor("wv", [128, CC, 128], bf, kind="ExternalInput")
    bqkv_d = nc.dram_tensor("bqkv", [128, NQK + 1], f32, kind="ExternalInput")
    cos_d = nc.dram_tensor("cosT", [128, T], bf, kind="ExternalInput")
    sin_d = nc.dram_tensor("sinT", [128, T], bf, kind="ExternalInput")
    swap_d = nc.dram_tensor("swp", [128, 128], bf, kind="ExternalInput")
    iden_d = nc.dram_tensor("idn", [128, 128], bf, kind="ExternalInput")
    wout_d = nc.dram_tensor("wout", [128, GQ, C], bf, kind="ExternalInput")
    out_d = nc.dram_tensor("out", [T, C], f32, kind="ExternalOutput")

    with tile.TileContext(nc) as tc:
        with (
            tc.tile_pool(name="persist", bufs=1) as pers,
            tc.tile_pool(name="xt", bufs=2) as xtp,
            tc.tile_pool(name="stage", bufs=4) as stg,
            tc.tile_pool(name="ptile", bufs=6) as ptp,
            tc.tile_pool(name="pacc", bufs=2) as pap,
            tc.tile_pool(name="small", bufs=2) as smp,
            tc.tile_pool(name="osb", bufs=3) as osp,
            tc.tile_pool(name="ps_a", bufs=3, space="PSUM") as ppa,
            tc.tile_pool(name="ps_y", bufs=1, space="PSUM") as ppy,
            tc.tile_pool(name="ps_o", bufs=3, space="PSUM") as ppo,
        ):
            import contextlib
            loop_cm = tc.For_i(0, reps, 1) if reps > 1 else contextlib.nullcontext()
            with loop_cm:
                _body(nc, tc, mybir, bf, f32, f8,
                      pers, xtp, stg, ptp, pap, smp, osp, ppa, ppy, ppo,
                      xt8_d, xtb_d, wqk8_d, wv_d, bqkv_d, cos_d, sin_d, swap_d,
                      iden_d, wout_d, out_d)
    nc.compile()
    return nc


def _body(nc, tc, mybir, bf, f32, f8,
          pers, xtp, stg, ptp, pap, smp, osp, ppa, ppy, ppo,
          xt8_d, xtb_d, wqk8_d, wv_d, bqkv_d, cos_d, sin_d, swap_d,
          iden_d, wout_d, out_d):
    Exp = mybir.ActivationFunctionType.Exp
    Ident = mybir.ActivationFunctionType.Identity
    DR = mybir.MatmulPerfMode.DoubleRow
    mul = mybir.AluOpType.mult
    add = mybir.AluOpType.add

    # ---- persistent loads, in need-order ----
    wqk_sb = pers.tile([128, CC, NQK * 128], f8)
    xt8_0 = xtp.tile([128, CC, 512], f8, tag="xt8")
    # tiny first chunk so the first matmul can start early, then two big ones
    for q in (slice(0, 2), slice(2, 9), slice(9, 16)):
        nc.sync.dma_start(wqk_sb[:, q, :], wqk8_d[:, q, :])
        nc.sync.dma_start(xt8_0[:, q, :], xt8_d[:, 0, q, :])
    bq_sb = pers.tile([128, NQK + 1], f32)
    nc.sync.dma_start(bq_sb[:], bqkv_d[:])
    swap_sb = pers.tile([128, 128], bf)
    nc.sync.dma_start(swap_sb[:], swap_d[:])
    iden_sb = pers.tile([128, 128], bf)
    nc.sync.dma_start(iden_sb[:], iden_d[:])
    cos_sb = pers.tile([128, T], bf)
    nc.sync.dma_start(cos_sb[:], cos_d[:])
    sin_sb = pers.tile([128, T], bf)
    nc.sync.dma_start(sin_sb[:], sin_d[:])
    wv_sb = pers.tile([128, CC, 128], bf)
    nc.sync.dma_start(wv_sb[:], wv_d[:])
    xtb_0 = xtp.tile([128, CC, 512], bf, tag="xtb")
    nc.sync.dma_start(xtb_0[:], xtb_d[:, 0])
    wout_sb = pers.tile([128, GQ, C], bf)
    nc.sync.dma_start(wout_sb[:], wout_d[:])
    ones_sb = pers.tile([128, 128], bf)
    nc.vector.memset(ones_sb[:], 1.0)

    # persistent activations
    qk_sb = pers.tile([128, NQK, T], bf)  # rotated q0..q3, k
    v_sb = pers.tile([128, TT, 128], bf)  # v in [t-part, d] tiles
    y_sb = pers.tile([128, GQ, T], bf)  # y^T per head

    # ---- phase 1: QKV + RoPE + v transpose ----
    rope_q = []  # deferred swap-matmul chains: (raw, f, tsl)

    def emit_rope(raw, f, tsl):
        psw = ppo.tile([128, 512], f32, tag="ops")
        nc.tensor.matmul(psw[:], swap_sb[:], raw[:], start=True, stop=True)
        tmp = stg.tile([128, 512], bf, tag="ropetmp")
        nc.vector.tensor_tensor(tmp[:], psw[:], sin_sb[:, tsl], mul)
        nc.vector.tensor_tensor(qk_sb[:, f, tsl], raw[:], cos_sb[:, tsl], mul)
        nc.vector.tensor_tensor(qk_sb[:, f, tsl], qk_sb[:, f, tsl], tmp[:], add)

    xt_next = (xt8_0, xtb_0)
    for ts in range(TS):
        tsl = slice(ts * 512, (ts + 1) * 512)
        xt8, xtb = xt_next
        if ts + 1 < TS:
            # prefetch the next slice now, ahead of this slice's v-transposes
            # in the sync queue, so the transfer overlaps this slice's compute
            nxt8 = xtp.tile([128, CC, 512], f8, tag="xt8")
            nc.sync.dma_start(nxt8[:], xt8_d[:, ts + 1])
            nxtb = xtp.tile([128, CC, 512], bf, tag="xtb")
            nc.sync.dma_start(nxtb[:], xtb_d[:, ts + 1])
            xt_next = (nxt8, nxtb)
        for f in range(NQK):
            ps = ppa.tile([128, 512], f32, tag="aps")
            for c2 in range(CC // 2):
                nc.tensor.matmul(
                    ps[:],
                    wqk_sb[:, 2 * c2 : 2 * c2 + 2, f * 128 : (f + 1) * 128],
                    xt8[:, 2 * c2 : 2 * c2 + 2, :],
                    start=(c2 == 0),
                    stop=(c2 == CC // 2 - 1),
                    perf_mode=DR,
                )
            # PSUM drain + un-scale + bias, bf16 out
            raw = stg.tile([128, 512], bf, tag="raw")
            nc.scalar.activation(
                raw[:], ps[:], Ident,
                bias=bq_sb[:, f : f + 1], scale=1.0 / WSCALE,
            )
            if rope_q:
                emit_rope(*rope_q.pop(0))
            rope_q.append((raw, f, tsl))
        # v: bf16 matmul
        ps = ppa.tile([128, 512], f32, tag="aps")
        for cc in range(CC):
            nc.tensor.matmul(
                ps[:],
                wv_sb[:, cc, :],
                xtb[:, cc, :],
                start=(cc == 0),
                stop=(cc == CC - 1),
            )
        vraw = stg.tile([128, 512], bf, tag="raw")
        nc.scalar.activation(
            vraw[:], ps[:], Ident, bias=bq_sb[:, NQK : NQK + 1], scale=1.0
        )
        if rope_q:
            emit_rope(*rope_q.pop(0))
        # v transpose [d, t] -> [t, d] via PE into one PSUM tile, single copy
        pvt = ppo.tile([128, 512], bf, tag="ops")
        for k in range(4):
            nc.tensor.transpose(
                pvt[:, k * 128 : (k + 1) * 128],
                vraw[:, k * 128 : (k + 1) * 128],
                iden_sb[:],
            )
        nc.vector.tensor_copy(v_sb[:, ts * 4 : ts * 4 + 4, :], pvt[:])
    while rope_q:
        emit_rope(*rope_q.pop(0))

    # ---- phase 2+3: attention (h-outer, pipelined av) with the previous
    # slice's out-projection sprinkled one PSUM-group per iteration ----
    PIPE = 3  # av/den lag this many score tiles behind

    def outproj_group(tt, es):
        pso = ppo.tile([128, 512], f32, tag="ops")
        for h in range(GQ):
            nc.tensor.matmul(
                pso[:],
                y_sb[:, h, tt * 128 : (tt + 1) * 128],
                wout_sb[:, h, es * 512 : (es + 1) * 512],
                start=(h == 0),
                stop=(h == GQ - 1),
            )
        o_sb = osb_cur[0]
        osl = slice(es * 512, (es + 1) * 512)
        if es % 2 == 0:
            nc.vector.tensor_copy(o_sb[:, osl], pso[:])
        else:
            nc.scalar.copy(o_sb[:, osl], pso[:])
        nc.sync.dma_start(out_d[tt * 128 : (tt + 1) * 128, osl], o_sb[:, osl])

    osb_cur = [None]

    def next_outproj_work(s):
        # yields (tt, es) pairs for slice s's out-projection
        for tt in range(4 * s, 4 * s + 4):
            osb_cur[0] = osp.tile([128, C], f32, tag="osb", name="osb")
            for es in range(4):
                yield tt, es

    for s in range(TS):
        isl = slice(s * 512, (s + 1) * 512)
        njt = 4 * (s + 1)
        pacc = [
            pap.tile([128, 512], bf, tag=f"pacc{h}", name=f"pacc{h}")
            for h in range(GQ)
        ]
        op_iter = iter(next_outproj_work(s - 1)) if s > 0 else None
        it = 0  # iteration count within this slice
        # spread the 16 out-proj groups across this slice's iterations
        op_stride = max(1, (16 * (s + 1) - 6) // 16)

        for h in range(GQ):
            psy = ppy.tile([128, 512], f32, tag=f"yps{h % 2}")
            pend = []  # (P tile, jt, off)

            acc_eng = nc.vector

            def drain_one():
                P, jt, off = pend.pop(0)
                nc.tensor.matmul(
                    psy[:, off:512],
                    v_sb[:, jt, :],
                    P[:, off:512],
                    start=(jt == 0),
                    stop=(jt == njt - 1),
                )
                if jt == 0:
                    acc_eng.tensor_copy(pacc[h][:], P[:])
                else:
                    acc_eng.tensor_tensor(
                        pacc[h][:, off:512], pacc[h][:, off:512], P[:, off:512], add
                    )

            for jt in range(njt):
                off = max(0, 128 * jt - 512 * s)
                pss = ppa.tile([128, 512], f32, tag="aps")
                nc.tensor.matmul(
                    pss[:, off:512],
                    qk_sb[:, GQ, jt * 128 : (jt + 1) * 128],
                    qk_sb[:, h, s * 512 + off : (s + 1) * 512],
                    start=True,
                    stop=True,
                )
                P = ptp.tile([128, 512], bf, tag="P")
                nc.scalar.activation(
                    P[:, off:512], pss[:, off:512], Exp, scale=1.0 / 128.0
                )
                if jt >= 4 * s:
                    nc.gpsimd.affine_select(
                        out=P[:, off : off + 128],
                        in_=P[:, off : off + 128],
                        pattern=[[1, 128]],
                        compare_op=mybir.AluOpType.is_ge,
                        fill=0.0,
                        base=0,
                        channel_multiplier=-1,
                    )
                while len(pend) >= PIPE:
                    drain_one()
                pend.append((P, jt, off))
                # sprinkle out-proj PSUM groups across the slice (from it=3,
                # giving the previous slice's last y-normalize time to land)
                if op_iter is not None and it >= 3 and (it - 3) % op_stride == 0:
                    for tt_es in op_iter:
                        outproj_group(*tt_es)
                        break
                it += 1
            while pend:
                drain_one()

            # normalize: den broadcast over partitions via ones-matmul,
            # then y = psy * (1/den)
            psd = ppo.tile([128, 512], f32, tag="ops")
            nc.tensor.matmul(psd[:], ones_sb[:], pacc[h][:], start=True, stop=True)
            rinv = smp.tile([128, 512], f32, tag="rinv")
            nc.vector.reciprocal_approx_fast(rinv[:], psd[:])
            nc.vector.tensor_tensor(y_sb[:, h, isl], psy[:], rinv[:], mul)

        if op_iter is not None:
            for tt_es in op_iter:
                outproj_group(*tt_es)

    for tt_es in next_outproj_work(TS - 1):
        outproj_group(*tt_es)


def _host_prep(x, rope_cache, W_qkv, b_qkv, W_out):
    """Build the 8 per-core input dicts."""
    import concourse.mybir as mybir

    F8 = mybir.dt.np(mybir.dt.float8e4)
    q_dim = NH * D  # 2048
    kv_dim = NKV * D  # 512

    # rope tables in [d, t] layout
    sin = rope_cache[:, 0::2].astype(np.float32)  # [T, 64]
    cos = rope_cache[:, 1::2].astype(np.float32)
    cos2T = np.empty((128, T), np.float32)
    sinsT = np.empty((128, T), np.float32)
    cos2T[0::2] = cos.T
    cos2T[1::2] = cos.T
    sinsT[0::2] = -sin.T
    sinsT[1::2] = sin.T
    cos2T = cos2T.astype(BF16)
    sinsT = sinsT.astype(BF16)

    swap = np.zeros((128, 128), BF16)
    idx = np.arange(128)
    swap[idx, idx ^ 1] = 1
    iden = np.eye(128, dtype=BF16)

    def tile_cols(w, ncols):
        # [C, ncols*128] -> [128, CC, ncols*128] with contraction c = cc*128+p
        return np.ascontiguousarray(
            w.reshape(CC, 128, ncols * 128).transpose(1, 0, 2)
        )

    in_maps = []
    for b in range(B):
        xT = np.ascontiguousarray(x[b].T)  # [C, T] f32
        xT = xT.reshape(CC, 128, T).transpose(1, 0, 2)  # [128, CC, T]
        # -> [128, TS, CC, 512]: per-slice DMA reads contiguous lines
        xT = xT.reshape(128, CC, TS, 512).transpose(0, 2, 1, 3)
        xt8 = np.ascontiguousarray(np.clip(xT, -240, 240).astype(F8))
        xtb = np.ascontiguousarray(xT.astype(BF16))
        for g in range(NKV):
            qk_cols = np.concatenate(
                [
                    np.arange(4 * g * D, (4 * g + 4) * D),  # 4 q heads
                    np.arange(q_dim + g * D, q_dim + (g + 1) * D),  # k head
                ]
            )
            v_cols = np.arange(q_dim + kv_dim + g * D, q_dim + kv_dim + (g + 1) * D)
            wqk = np.clip(W_qkv[:, qk_cols] * WSCALE, -240, 240).astype(F8)
            wv = W_qkv[:, v_cols].astype(BF16)
            bq = np.ascontiguousarray(
                b_qkv[np.concatenate([qk_cols, v_cols])]
                .astype(np.float32).reshape(NQK + 1, 128).T
            )  # [128, NQK+1]
            wo = W_out[4 * g * D : (4 * g + 4) * D, :].astype(BF16)  # [512, C]
            wo = np.ascontiguousarray(
                wo.reshape(GQ, 128, C).transpose(1, 0, 2)
            )  # [128, GQ, C]
            in_maps.append(
                {
                    "xt8": xt8,
                    "xtb": xtb,
                    "wqk8": tile_cols(wqk, NQK),
                    "wv": tile_cols(wv, 1),
                    "bqkv": bq,
                    "cosT": cos2T,
                    "sinT": sinsT,
                    "swp": swap,
                    "idn": iden,
                    "wout": wo,
                }
            )
    return in_maps


def kernel(x, rope_cache, W_qkv, b_qkv, W_out, b_out, _trace=False):
    from concourse.bass_utils import run_bass_kernel_spmd

    if "nc" not in _CACHED:
        _CACHED["nc"] = _build_bass()
    nc = _CACHED["nc"]

    in_maps = _host_prep(
        np.asarray(x), np.asarray(rope_cache), np.asarray(W_qkv),
        np.asarray(b_qkv), np.asarray(W_out),
    )
    res = run_bass_kernel_spmd(nc, in_maps, core_ids=list(range(N_CORES)), trace=_trace)
    _CACHED["last_result"] = res

    out = np.zeros((B, T, C), np.float32)
    for b in range(B):
        acc = res.results[b * NKV]["out"].astype(np.float32)
        for g in range(1, NKV):
            acc = acc + res.results[b * NKV + g]["out"]
        out[b] = acc + np.asarray(b_out)[None, :]
    return out
